# revision 6
# baseline (speedup 1.0000x reference)
"""Low-rank ray tracer CSI kernel for 8 Trainium2 NeuronCores.

Reference computation:
    A = einsum('dpr,kr->dk', ua, F); B = einsum('dpr,kr->dk', ub, F)
    csi[k] = sum_d A[d,k]*B[d,k] / D

Since F has no p index, A = (sum_p ua) @ F^T.  Let Ua[d,r] = sum_p ua[d,p,r]
(same for Ub).  Then
    csi[k] = (1/D) * sum_d (Ua F^T)[d,k] (Ub F^T)[d,k]
           = (1/D) * f_k^T (Ua^T Ub) f_k  =  (1/D) * f_k^T M f_k
with M = Ua^T Ub a tiny [R,R] Gram matrix.  Sharding d across cores makes M
additive, and csi is linear in M, so each core returns its partial csi and the
host sums 8 vectors of 4 KB.  The kernel is then purely DMA-bound: each core
streams its 16 MiB shard once; the only non-trivial compute is the p-reduction
on the vector engine, which hides under the DMA.
"""

import sys

import numpy as np

sys.path.insert(0, "/opt/trn_rl_repo")

import concourse.bacc as bacc
import concourse.bass as bass
import concourse.mybir as mybir
from concourse.bass_utils import run_bass_kernel_spmd
from concourse.masks import make_identity
from concourse.tile import TileContext

D, P, R, K = 1024, 256, 64, 1024
NCORES = 8
DC = D // NCORES  # directions per core
PC = 64  # p-chunk per DMA/reduce step
NCH = P // PC  # chunks per tensor
KC = K // 128  # k chunks of 128 (PSUM partition limit)

F32 = mybir.dt.float32


def build_bass() -> bass.Bass:
    nc = bacc.Bacc(None, target_bir_lowering=False)
    ua = nc.declare_dram_parameter("ua", [DC, P, R], F32, isOutput=False)
    ub = nc.declare_dram_parameter("ub", [DC, P, R], F32, isOutput=False)
    f = nc.declare_dram_parameter("f", [K, R], F32, isOutput=False)
    # out[p, c] = partial csi[c*128 + p], already scaled by 1/D
    out = nc.declare_dram_parameter("out", [128, KC], F32, isOutput=True)

    with TileContext(nc) as tc:
        with (
            tc.tile_pool(name="const", bufs=1) as cpool,
            tc.tile_pool(name="chunks", bufs=4) as chpool,
            tc.tile_pool(name="small", bufs=1) as spool,
            tc.tile_pool(name="scratch", bufs=2) as scpool,
            tc.tile_pool(name="psum", bufs=2, space="PSUM") as ppool,
            tc.tile_pool(name="psum1", bufs=1, space="PSUM") as ppool1,
        ):
            identity = cpool.tile([128, 128], F32)
            make_identity(nc, identity[:])

            # F in natural layout, k on partitions: [128, KC, R]
            f_sb = cpool.tile([128, KC, R], F32)
            nc.sync.dma_start(out=f_sb[:], in_=f.rearrange("(c p) r -> p c r", p=128))

            # F^T [R, K] via PE transposes of the natural chunks
            ft_sb = cpool.tile([R, K], F32)
            for c in range(KC):
                ftp = ppool.tile([R, 128], F32, tag="ftp")
                nc.tensor.transpose(ftp[:], f_sb[:, c, :], identity[:])
                nc.vector.tensor_copy(out=ft_sb[:, c * 128 : (c + 1) * 128], in_=ftp[:])

            # Streaming p-reduction: Ua[d,r] = sum_p ua[d,p,r] (same for ub)
            us = []
            for name, t_ap in (("a", ua), ("b", ub)):
                parts = spool.tile([DC, NCH, R], F32, tag=f"parts_{name}")
                for i in range(NCH):
                    ch = chpool.tile([DC, PC, R], F32, tag="chunk")
                    nc.sync.dma_start(out=ch[:], in_=t_ap[:, i * PC : (i + 1) * PC, :])
                    # view [d, r, p]: reduce innermost (p, stride R) per (d, r)
                    nc.vector.tensor_reduce(
                        out=parts[:, i, :],
                        in_=ch[:].transpose([0, 2, 1]),
                        axis=mybir.AxisListType.X,
                        op=mybir.AluOpType.add,
                    )
                u = spool.tile([DC, R], F32, tag=f"u_{name}")
                nc.vector.tensor_reduce(
                    out=u[:],
                    in_=parts[:].transpose([0, 2, 1]),
                    axis=mybir.AxisListType.X,
                    op=mybir.AluOpType.add,
                )
                us.append(u)

            # Gram matrix M[r1,r2] = sum_d Ua[d,r1] Ub[d,r2]
            m_psum = ppool1.tile([R, R], F32)
            nc.tensor.matmul(m_psum[:], us[0][:], us[1][:], start=True, stop=True)
            # fold the 1/D normalization into M while copying out of PSUM
            m_sb = spool.tile([R, R], F32)
            nc.scalar.mul(m_sb[:], m_psum[:], 1.0 / D)

            # csi[k] = (1/D) * sum_r2 (sum_r1 F[k,r1] M[r1,r2]) * F[k,r2]
            csi = spool.tile([128, KC], F32)
            for c in range(KC):
                g_psum = ppool.tile([128, R], F32, tag="g")
                nc.tensor.matmul(
                    g_psum[:],
                    ft_sb[:, c * 128 : (c + 1) * 128],
                    m_sb[:],
                    start=True,
                    stop=True,
                )
                scr = scpool.tile([128, R], F32, tag="scr")
                nc.vector.tensor_mul(out=scr[:], in0=g_psum[:], in1=f_sb[:, c, :])
                nc.vector.tensor_reduce(
                    out=csi[:, c : c + 1],
                    in_=scr[:],
                    axis=mybir.AxisListType.X,
                    op=mybir.AluOpType.add,
                )
            nc.sync.dma_start(out=out[:], in_=csi[:])
    nc.compile()
    return nc


_NC_CACHE = None


def kernel(**inputs: np.ndarray) -> np.ndarray:
    global _NC_CACHE
    ua = np.ascontiguousarray(inputs["attenuation_vectors"], dtype=np.float32)
    ub = np.ascontiguousarray(inputs["radiation_vectors"], dtype=np.float32)
    f = np.ascontiguousarray(inputs["frequency_basis_vectors"], dtype=np.float32)

    if _NC_CACHE is None:
        _NC_CACHE = build_bass()
    nc = _NC_CACHE

    in_maps = [
        {"ua": ua[c * DC : (c + 1) * DC], "ub": ub[c * DC : (c + 1) * DC], "f": f}
        for c in range(NCORES)
    ]
    res = run_bass_kernel_spmd(nc, in_maps, list(range(NCORES)))
    acc = np.zeros((128, KC), dtype=np.float32)
    for r in res.results:
        acc += r["out"]
    return acc.T.reshape(K).astype(np.float32)


if __name__ == "__main__":
    rng = np.random.default_rng(0)
    ins = {
        "attenuation_vectors": rng.standard_normal((D, P, R), dtype=np.float32),
        "radiation_vectors": rng.standard_normal((D, P, R), dtype=np.float32),
        "frequency_basis_vectors": rng.standard_normal((K, R), dtype=np.float32),
    }
    got = kernel(**ins)
    ua_s = ins["attenuation_vectors"].sum(axis=1)
    ub_s = ins["radiation_vectors"].sum(axis=1)
    a = ua_s @ ins["frequency_basis_vectors"].T
    b = ub_s @ ins["frequency_basis_vectors"].T
    want = (a * b).sum(axis=0) / D
    err = np.abs(got - want).max() / np.abs(want).max()
    print("rel err vs local numpy:", err)


# revision 7
# speedup vs baseline: 1.1730x; 1.1730x over previous
"""Low-rank ray tracer CSI kernel for 8 Trainium2 NeuronCores.

Reference computation:
    A = einsum('dpr,kr->dk', ua, F); B = einsum('dpr,kr->dk', ub, F)
    csi[k] = sum_d A[d,k]*B[d,k] / D

Since F has no p index, A = (sum_p ua) @ F^T.  Let Ua[d,r] = sum_p ua[d,p,r]
(same for Ub).  Then
    csi[k] = (1/D) * sum_d (Ua F^T)[d,k] (Ub F^T)[d,k]
           = (1/D) * f_k^T (Ua^T Ub) f_k  =  (1/D) * f_k^T M f_k
with M = Ua^T Ub a tiny [R,R] Gram matrix.  Sharding d across cores makes M
additive, and csi is linear in M, so each core returns its partial csi and the
host sums 8 vectors of 4 KB.  The kernel is then purely DMA-bound: each core
streams its 16 MiB shard once; the only non-trivial compute is the p-reduction
on the vector engine, which hides under the DMA.

The host pre-transposes the inputs to [D, R, P] so that the p axis is
contiguous in SBUF: the vector-engine reduce then runs with a stride-1 inner
axis (single-src perf mode) instead of the 4x-slower strided form, and each
chunk reduce writes its Ua columns directly (no second reduction stage).
"""

import sys

import numpy as np

sys.path.insert(0, "/opt/trn_rl_repo")

import concourse.bacc as bacc
import concourse.bass as bass
import concourse.mybir as mybir
from concourse.bass_utils import run_bass_kernel_spmd
from concourse.masks import make_identity
from concourse.tile import TileContext

D, P, R, K = 1024, 256, 64, 1024
NCORES = 8
DC = D // NCORES  # directions per core
RC = 16  # r-chunk per DMA/reduce step (input layout [D, R, P])
NCH = R // RC  # chunks per tensor
KC = K // 128  # k chunks of 128 (PSUM partition limit)

F32 = mybir.dt.float32


def build_bass() -> bass.Bass:
    nc = bacc.Bacc(None, target_bir_lowering=False)
    # per-core shards, pre-transposed to [d, r, p]
    ua = nc.declare_dram_parameter("ua", [DC, R, P], F32, isOutput=False)
    ub = nc.declare_dram_parameter("ub", [DC, R, P], F32, isOutput=False)
    f = nc.declare_dram_parameter("f", [K, R], F32, isOutput=False)
    # out[p, c] = partial csi[c*128 + p], already scaled by 1/D
    out = nc.declare_dram_parameter("out", [128, KC], F32, isOutput=True)

    with TileContext(nc) as tc:
        with (
            tc.tile_pool(name="const", bufs=1) as cpool,
            tc.tile_pool(name="chunks", bufs=2 * NCH) as chpool,
            tc.tile_pool(name="small", bufs=1) as spool,
            tc.tile_pool(name="scratch", bufs=2) as scpool,
            tc.tile_pool(name="psum", bufs=2, space="PSUM") as ppool,
            tc.tile_pool(name="psum1", bufs=1, space="PSUM") as ppool1,
        ):
            identity = cpool.tile([128, 128], F32)
            make_identity(nc, identity[:])

            # F in natural layout, k on partitions: [128, KC, R]
            f_sb = cpool.tile([128, KC, R], F32)
            nc.sync.dma_start(out=f_sb[:], in_=f.rearrange("(c p) r -> p c r", p=128))

            # F^T [R, K] via PE transposes of the natural chunks
            ft_sb = cpool.tile([R, K], F32)
            for c in range(KC):
                ftp = ppool.tile([R, 128], F32, tag="ftp")
                nc.tensor.transpose(ftp[:], f_sb[:, c, :], identity[:])
                nc.vector.tensor_copy(out=ft_sb[:, c * 128 : (c + 1) * 128], in_=ftp[:])

            # Streaming p-reduction: Ua[d,r] = sum_p ua[d,r,p] (same for ub)
            us = []
            for name, t_ap in (("a", ua), ("b", ub)):
                u = spool.tile([DC, R], F32, tag=f"u_{name}")
                for i in range(NCH):
                    ch = chpool.tile([DC, RC, P], F32, tag="chunk")
                    nc.sync.dma_start(out=ch[:], in_=t_ap[:, i * RC : (i + 1) * RC, :])
                    nc.vector.tensor_reduce(
                        out=u[:, i * RC : (i + 1) * RC],
                        in_=ch[:],
                        axis=mybir.AxisListType.X,
                        op=mybir.AluOpType.add,
                    )
                us.append(u)

            # Gram matrix M[r1,r2] = sum_d Ua[d,r1] Ub[d,r2]
            m_psum = ppool1.tile([R, R], F32)
            nc.tensor.matmul(m_psum[:], us[0][:], us[1][:], start=True, stop=True)
            # fold the 1/D normalization into M while copying out of PSUM
            m_sb = spool.tile([R, R], F32)
            nc.scalar.mul(m_sb[:], m_psum[:], 1.0 / D)

            # csi[k] = sum_r2 (sum_r1 F[k,r1] (M/D)[r1,r2]) * F[k,r2]
            csi = spool.tile([128, KC], F32)
            for c in range(KC):
                g_psum = ppool.tile([128, R], F32, tag="g")
                nc.tensor.matmul(
                    g_psum[:],
                    ft_sb[:, c * 128 : (c + 1) * 128],
                    m_sb[:],
                    start=True,
                    stop=True,
                )
                scr = scpool.tile([128, R], F32, tag="scr")
                nc.vector.tensor_mul(out=scr[:], in0=g_psum[:], in1=f_sb[:, c, :])
                nc.vector.tensor_reduce(
                    out=csi[:, c : c + 1],
                    in_=scr[:],
                    axis=mybir.AxisListType.X,
                    op=mybir.AluOpType.add,
                )
            nc.sync.dma_start(out=out[:], in_=csi[:])
    nc.compile()
    return nc


_NC_CACHE = None


def kernel(**inputs: np.ndarray) -> np.ndarray:
    global _NC_CACHE
    ua = np.asarray(inputs["attenuation_vectors"], dtype=np.float32)
    ub = np.asarray(inputs["radiation_vectors"], dtype=np.float32)
    f = np.ascontiguousarray(inputs["frequency_basis_vectors"], dtype=np.float32)

    # [D, P, R] -> [D, R, P] so the p axis is contiguous on-device
    ua_t = np.ascontiguousarray(ua.transpose(0, 2, 1))
    ub_t = np.ascontiguousarray(ub.transpose(0, 2, 1))

    if _NC_CACHE is None:
        _NC_CACHE = build_bass()
    nc = _NC_CACHE

    in_maps = [
        {
            "ua": ua_t[c * DC : (c + 1) * DC],
            "ub": ub_t[c * DC : (c + 1) * DC],
            "f": f,
        }
        for c in range(NCORES)
    ]
    res = run_bass_kernel_spmd(nc, in_maps, list(range(NCORES)))
    acc = np.zeros((128, KC), dtype=np.float32)
    for r in res.results:
        acc += r["out"]
    return acc.T.reshape(K).astype(np.float32)


if __name__ == "__main__":
    rng = np.random.default_rng(0)
    ins = {
        "attenuation_vectors": rng.standard_normal((D, P, R), dtype=np.float32),
        "radiation_vectors": rng.standard_normal((D, P, R), dtype=np.float32),
        "frequency_basis_vectors": rng.standard_normal((K, R), dtype=np.float32),
    }
    got = kernel(**ins)
    ua_s = ins["attenuation_vectors"].sum(axis=1)
    ub_s = ins["radiation_vectors"].sum(axis=1)
    a = ua_s @ ins["frequency_basis_vectors"].T
    b = ub_s @ ins["frequency_basis_vectors"].T
    want = (a * b).sum(axis=0) / D
    err = np.abs(got - want).max() / np.abs(want).max()
    print("rel err vs local numpy:", err)
